# revision 2
# baseline (speedup 1.0000x reference)
"""Fused transformer encoder layer (attention w/ 2D-ALiBi bias + FFN) on 8 trn2 cores.

Sharding: core c handles batch b = c//2, token half h = c%2 (512 query rows).
K/V are computed per-core for the full 1024-token sequence of its batch
(duplicated across the 2 cores sharing a batch); outputs are disjoint row
slices of the final tensor, so no collectives are needed.

Bias trick: the alibi_2d bias slope_h*(|xi-xj|+|yi-yj|) is folded into the
QK^T contraction. |xi-xj| = xi + xj - 2*a_i.a_j with a_i in {0,1}^31 the
threshold indicators of xi, so dist(i,j) = s_i + s_j - 2*c_i.c_j (c = 62-dim
indicator, s = x+y). The per-query term slope*s_i is constant along the
softmax axis and is dropped. Q/K are augmented with 64 extra contraction dims
(s_j / pad / c_j on the K side; 1 / 0 / -2*c_i on the Q side), making the
score contraction K = 64+64 = 128 exactly — full PE array, bias for free.

bf16 precision care: the aug rows are small integers / {0,-2} — exact in
bf16. The attention scale AND the per-head slope are folded out of the bf16
data: Q-projection weights carry scale/slope_h per head (so scores come out
as S/slope_h) and the exact fp32 slope_h is re-applied as the exp()
activation's scale immediate. exp needs no max-subtraction (|S| <= ~50 by
construction).

v2 attention dataflow: scores are computed keys-on-partitions (S^T) so exp()
output P^T feeds the AV matmuls as the STATIONARY operand (chunks [128k,128q])
with V as the 65-wide moving operand (64 dims + a ones column that yields the
softmax denominator). This produces O in [query, head-dim] layout at 65 moving
cols per (head, q-chunk, key-chunk) instead of 512 — ~2x less PE time in AV —
and makes the softmax normalization a plain per-partition (per-query)
tensor_scalar multiply. Normalized O tiles are PE-transposed back to O^T for
the head-summing out-projection. The V projection is computed per head-pair
and interleaved into the attention head loop, filling PE slack under the
ACT-bound exp stream.
"""

import math
import sys
import time

for _p in ("/opt/trn_rl_repo",):
    if _p not in sys.path:
        sys.path.insert(0, _p)

import numpy as np
import ml_dtypes

import concourse.bass as bass
import concourse.tile as tile
from concourse import bacc, mybir
from concourse.masks import make_identity

F32 = mybir.dt.float32
BF16 = mybir.dt.bfloat16
BF = ml_dtypes.bfloat16

D = 1024          # d_model
H = 16            # heads
HD = 64           # head dim
DFF = 4096
B = 4
N = 1024          # sequence length
NT = 512          # tokens (query rows) per core
GRID = 32
EPS = 1e-5
NCORES = 8
SCALE = HD ** -0.5


def _alibi_slopes(n):
    def pow2(n_):
        start = 2.0 ** (-(2.0 ** -(math.log2(n_) - 3)))
        return [start * start ** i for i in range(n_)]
    if math.log2(n).is_integer():
        return np.array(pow2(n), dtype=np.float64)
    m = 2 ** math.floor(math.log2(n))
    s = pow2(m)
    s += [s[-1] * 0.5 ** (i + 1) for i in range(n - m)]
    return np.array(s, dtype=np.float64)


SLOPES = _alibi_slopes(H)


def build_nc(trivial_affine=False):
    """trivial_affine: g1/g2 all-ones and be1/be2/b2 all-zeros -> skip those ops."""
    nc = bacc.Bacc()

    # srcT columns are host-permuted per core: own query half first, so the
    # Q projection reads stf[:, :, 0:512] (keys/V follow the same order —
    # softmax is order-invariant as long as kaug_x matches).
    srcT = nc.declare_dram_parameter("srcT", [D, N], BF16, isOutput=False)
    src_rows = nc.declare_dram_parameter("src_rows", [NT, D], F32, isOutput=False)
    # W{q,k,v}S[bt, p, dc*128+j] = W*T[dc*128+p, bt*128+j]: each 128-wide
    # output block is one fully-contiguous 256KB DMA (2KB runs/partition).
    WqS = nc.declare_dram_parameter("WqS", [8, 128, D], BF16, isOutput=False)
    WkS = nc.declare_dram_parameter("WkS", [8, 128, D], BF16, isOutput=False)
    WvS = nc.declare_dram_parameter("WvS", [8, 128, D], BF16, isOutput=False)
    WoT = nc.declare_dram_parameter("WoT", [D, D], BF16, isOutput=False)
    # W1 pre-swizzled on host: W1S[ft, p, dc*128+j] = W1.T[dc*128+p, ft*128+j]
    # so each FFN1 weight block is one fully-contiguous 256KB DMA.
    W1S = nc.declare_dram_parameter("W1S", [32, 128, D], BF16, isOutput=False)
    W2Tb = nc.declare_dram_parameter("W2Tb", [DFF, D], BF16, isOutput=False)
    kaug_x = nc.declare_dram_parameter("kaug_x", [64, N], BF16, isOutput=False)
    qaug_x = nc.declare_dram_parameter("qaug_x", [H, 64, NT], BF16, isOutput=False)
    b1r = nc.declare_dram_parameter("b1r", [128, 32], F32, isOutput=False)
    b2 = nc.declare_dram_parameter("b2", [1, D], F32, isOutput=False)
    g1 = nc.declare_dram_parameter("g1", [1, D], F32, isOutput=False)
    be1 = nc.declare_dram_parameter("be1", [1, D], F32, isOutput=False)
    g2 = nc.declare_dram_parameter("g2", [1, D], F32, isOutput=False)
    be2 = nc.declare_dram_parameter("be2", [1, D], F32, isOutput=False)
    out = nc.declare_dram_parameter("out", [NT, D], BF16, isOutput=True)

    AF = mybir.ActivationFunctionType
    OP = mybir.AluOpType

    with tile.TileContext(nc) as tc:
        with (
            tc.tile_pool(name="misc", bufs=1) as misc,
            tc.tile_pool(name="lnp", bufs=4) as lnp,
        ):
            eps_sb = misc.tile([128, 1], F32, tag="eps")
            nc.vector.memset(eps_sb, EPS)
            ident = misc.tile([128, 128], F32, tag="ident")
            make_identity(nc, ident)
            identb = misc.tile([128, 128], BF16, tag="identb")
            make_identity(nc, identb)
            # OT_sb[p, c, q]: head 2c in partitions 0:64, head 2c+1 in 64:128
            OT_sb = misc.tile([128, 8, NT], BF16, tag="otsb")
            # wof/srar live in the outer scope so their DMAs don't alias
            # (and thus wait on) attention SBUF: issued mid-attention, ready
            # well before the out-projection needs them.
            wof = misc.tile([128, 8, D], BF16, tag="wof")
            srar = misc.tile([128, 4, D], F32, tag="srcrows")

            def ln_apply(x_ap, gbc, bbc):
                stats = lnp.tile([128, 2, 6], F32, tag="lnstats", name="lnstats")
                for sg in range(2):
                    nc.vector.bn_stats(
                        out=stats[:, sg, :], in_=x_ap[:, sg * 512 : sg * 512 + 512]
                    )
                mv = lnp.tile([128, 2], F32, tag="lnmv", name="lnmv")
                nc.vector.bn_aggr(out=mv, in_=stats)
                nc.scalar.activation(
                    out=mv[:, 1:2], in_=mv[:, 1:2], func=AF.Sqrt,
                    bias=eps_sb, scale=1.0,
                )
                nc.vector.reciprocal(out=mv[:, 1:2], in_=mv[:, 1:2])
                nc.vector.tensor_scalar(
                    out=x_ap, in0=x_ap,
                    scalar1=mv[:, 0:1], scalar2=mv[:, 1:2],
                    op0=OP.subtract, op1=OP.mult,
                )
                if gbc is not None:
                    nc.vector.tensor_mul(out=x_ap, in0=x_ap, in1=gbc)
                if bbc is not None:
                    nc.vector.tensor_add(out=x_ap, in0=x_ap, in1=bbc)

            # ============ attention scope ============
            with tc.tile_pool(name="att", bufs=1) as att:
                kaug = att.tile([128, H, N], BF16, tag="kaug")
                qaug = att.tile([128, H, NT], BF16, tag="qaug")
                # v_sb[p, mt, h, j]: V[token mt*128+p, head h dim j]; j=64 is
                # the ones column (softmax denominator via AV contraction).
                v_sb = att.tile([128, 8, H, 65], BF16, tag="vsb")
                nc.vector.memset(v_sb[:, :, :, 64], 1.0)
                # stf/wvf live for the whole attention scope: the V projection
                # for pairs 1-7 is interleaved into the head loop below.
                stf = att.tile([128, 8, N], BF16, tag="stf")
                wvf = att.tile([128, 8, D], BF16, tag="wvf")

                # --- phase 1: Q/K projections + V proj for pair 0 ---
                with tc.tile_pool(name="ph1", bufs=1) as ph1:
                    # fine-grained loads in consumption order so the PE never
                    # outruns the (serialized) DMA stream by much
                    st_vw = srcT[:, :].rearrange("(c p) n -> p c n", p=128)
                    wqf = ph1.tile([128, 8, D], BF16, tag="wqf")
                    wkf = ph1.tile([128, 8, D], BF16, tag="wkf")
                    nc.sync.dma_start(out=stf[:, 0, 0:NT], in_=st_vw[:, 0, 0:NT])
                    nc.sync.dma_start(out=wqf[:, 0, 0:128], in_=WqS[0, :, 0:128])
                    nc.sync.dma_start(out=stf[:, 1, 0:NT], in_=st_vw[:, 1, 0:NT])
                    nc.sync.dma_start(out=wqf[:, 0, 128:D], in_=WqS[0, :, 128:D])
                    nc.sync.dma_start(out=stf[:, 2, 0:NT], in_=st_vw[:, 2, 0:NT])
                    nc.sync.dma_start(out=stf[:, 3, 0:NT], in_=st_vw[:, 3, 0:NT])
                    nc.sync.dma_start(out=stf[:, 4:6, 0:NT], in_=st_vw[:, 4:6, 0:NT])
                    nc.sync.dma_start(out=stf[:, 6:8, 0:NT], in_=st_vw[:, 6:8, 0:NT])
                    for dt in range(1, 8):
                        nc.sync.dma_start(out=wqf[:, dt, :], in_=WqS[dt, :, :])
                    nc.sync.dma_start(out=wkf[:, 0, :], in_=WkS[0, :, :])
                    nc.sync.dma_start(out=stf[:, 0:4, NT:N], in_=st_vw[:, 0:4, NT:N])
                    nc.sync.dma_start(out=stf[:, 4:8, NT:N], in_=st_vw[:, 4:8, NT:N])
                    for dt in range(1, 8):
                        nc.sync.dma_start(out=wkf[:, dt, :], in_=WkS[dt, :, :])
                    for p in range(8):
                        nc.sync.dma_start(out=wvf[:, p, :], in_=WvS[p, :, :])
                    for h in range(H):
                        nc.sync.dma_start(out=kaug[64:128, h, :], in_=kaug_x[:, :])
                        nc.sync.dma_start(out=qaug[64:128, h, :], in_=qaug_x[h, :, :])
                    # early prefetch for phase 3 (DMA engines idle mid-attention)
                    nc.sync.dma_start(
                        out=wof, in_=WoT[:, :].rearrange("(c p) n -> p c n", p=128)
                    )
                    nc.sync.dma_start(
                        out=srar,
                        in_=src_rows[:, :].rearrange("(nt p) d -> p nt d", p=128),
                    )

                    # Q + K projections -> qaug/kaug top halves (shared pool)
                    with tc.tile_pool(name="psQK", bufs=3, space="PSUM") as psQK:
                        for dt in range(8):
                            qps = psQK.tile([128, NT], F32, tag="proj", name="qps")
                            for dc in range(8):
                                nc.tensor.matmul(
                                    qps,
                                    wqf[:, dt, dc * 128 : dc * 128 + 128],
                                    stf[:, dc, 0:NT],
                                    start=(dc == 0), stop=(dc == 7),
                                )
                            nc.scalar.activation(
                                out=qaug[0:64, 2 * dt, :], in_=qps[0:64, :], func=AF.Copy
                            )
                            nc.vector.tensor_copy(
                                out=qaug[0:64, 2 * dt + 1, :], in_=qps[64:128, :]
                            )
                        for dt in range(8):
                            for mh in range(2):
                                kps = psQK.tile([128, 512], F32, tag="proj", name="kps")
                                for dc in range(8):
                                    nc.tensor.matmul(
                                        kps,
                                        wkf[:, dt, dc * 128 : dc * 128 + 128],
                                        stf[:, dc, mh * 512 : mh * 512 + 512],
                                        start=(dc == 0), stop=(dc == 7),
                                    )
                                nc.scalar.activation(
                                    out=kaug[0:64, 2 * dt, mh * 512 : mh * 512 + 512],
                                    in_=kps[0:64, :], func=AF.Copy,
                                )
                                nc.vector.tensor_copy(
                                    out=kaug[0:64, 2 * dt + 1, mh * 512 : mh * 512 + 512],
                                    in_=kps[64:128, :],
                                )
                        # V projection for pair 0 only (pairs 1-7 are
                        # interleaved into the attention head loop below)
                        for mt in range(8):
                            vp = psQK.tile([128, 128], F32, tag="vp1", name="vp")
                            for dc in range(8):
                                nc.tensor.matmul(
                                    vp,
                                    stf[:, dc, mt * 128 : mt * 128 + 128],
                                    wvf[:, 0, dc * 128 : dc * 128 + 128],
                                    start=(dc == 0), stop=(dc == 7),
                                )
                            vdst = v_sb[:, mt, 0:2, 0:64]
                            vsrc = vp.rearrange("p (h w) -> p h w", w=64)
                            if mt % 2 == 0:
                                nc.scalar.activation(out=vdst, in_=vsrc, func=AF.Copy)
                            else:
                                nc.vector.tensor_copy(out=vdst, in_=vsrc)

                # --- phase 2: attention head loop ---
                # step s: scores+exp(h=s) | AV+normalize(h=s-1) | transpose
                # (pair s//2-1 at even s) | V proj half-unit for pair s//2+1.
                with (
                    tc.tile_pool(name="ptp", bufs=4) as ptp,
                    tc.tile_pool(name="osbp", bufs=2) as osbp,
                    tc.tile_pool(name="recp", bufs=2) as recp,
                    tc.tile_pool(name="psS", bufs=2, space="PSUM") as psS,
                    tc.tile_pool(name="psAV", bufs=2, space="PSUM") as psAV,
                    tc.tile_pool(name="psTP", bufs=1, space="PSUM") as psTP,
                    tc.tile_pool(name="psV2", bufs=1, space="PSUM") as psV2,
                ):
                    pts = {}
                    avps = {}
                    osbs = {}

                    def v_chunk(p, mt):
                        vp = psV2.tile([128, 128], F32, tag="vp2", name="vp2")
                        for dc in range(8):
                            nc.tensor.matmul(
                                vp,
                                stf[:, dc, mt * 128 : mt * 128 + 128],
                                wvf[:, p, dc * 128 : dc * 128 + 128],
                                start=(dc == 0), stop=(dc == 7),
                            )
                        nc.vector.tensor_copy(
                            out=v_sb[:, mt, 2 * p : 2 * p + 2, 0:64],
                            in_=vp.rearrange("p (h w) -> p h w", w=64),
                        )

                    # AV(h) runs at step h+2 (not h+1): a full extra step of
                    # slack for the ACT exp stream to finish pt(h).
                    for step in range(H + 2):
                        h = step if step < H else None
                        hp = step - 2 if step >= 2 else None
                        # V-proj half-unit for pair step//2+1 (mts by parity)
                        vjobs = []
                        if step < 14:
                            pv = step // 2 + 1
                            vjobs = [(pv, (step % 2) * 4 + j) for j in range(4)]

                        if h is not None:
                            pt = ptp.tile([128, 8, NT], BF16, tag="pt", name="pt")
                            pts[h] = pt
                        if hp is not None:
                            avp = psAV.tile(
                                [128, 4, 128], F32, tag="avp", name="avp"
                            )
                            avps[hp] = avp

                        # scores g0/g1 + first half of AV(hp) + V work,
                        # then scores g2/g3, AV second half, transposes.
                        for g in range(4):
                            if h is not None:
                                stg = psS.tile(
                                    [128, 2, NT], F32, tag="stg", name="stg"
                                )
                                for j in range(2):
                                    mt = 2 * g + j
                                    nc.tensor.matmul(
                                        stg[:, j, :],
                                        kaug[:, h, mt * 128 : mt * 128 + 128],
                                        qaug[:, h, :],
                                        start=True, stop=True,
                                    )
                                nc.scalar.activation(
                                    out=pt[:, 2 * g : 2 * g + 2, :], in_=stg,
                                    func=AF.Exp, scale=float(SLOPES[h]),
                                )
                            if vjobs and g % 2 == 1:
                                for p, mt in vjobs[2 * (g // 2) : 2 * (g // 2) + 2]:
                                    v_chunk(p, mt)
                            if hp is not None:
                                ptc = pts[hp]
                                for qc in (g,):
                                    for mt in range(8):
                                        nc.tensor.matmul(
                                            avp[:, qc, 0:65],
                                            ptc[:, mt, qc * 128 : qc * 128 + 128],
                                            v_sb[:, mt, hp, :],
                                            start=(mt == 0), stop=(mt == 7),
                                        )

                        if hp is not None:
                            pts.pop(hp)
                            avp = avps.pop(hp)
                            c = hp // 2
                            if hp % 2 == 0:
                                osb = osbp.tile(
                                    [128, 4, 128], BF16, tag="osb", name="osb"
                                )
                                osbs[c] = osb
                            osb = osbs[c]
                            hcol = (hp % 2) * 64
                            rec = recp.tile([128, 4, 1], F32, tag="rec", name="rec")
                            nc.vector.reciprocal(out=rec, in_=avp[:, :, 64:65])
                            for qc in range(4):
                                nc.vector.tensor_scalar(
                                    out=osb[:, qc, hcol : hcol + 64],
                                    in0=avp[:, qc, 0:64],
                                    scalar1=rec[:, qc, :], scalar2=None,
                                    op0=OP.mult,
                                )
                            if hp % 2 == 1:
                                osb = osbs.pop(c)
                                for qc in range(4):
                                    tp = psTP.tile(
                                        [128, 128], BF16, tag="tp", name="tp"
                                    )
                                    nc.tensor.transpose(tp, osb[:, qc, :], identb)
                                    nc.vector.tensor_copy(
                                        out=OT_sb[:, c, qc * 128 : qc * 128 + 128],
                                        in_=tp,
                                    )

            # ============ post-attention scope ============
            with tc.tile_pool(name="ffn", bufs=1) as ffn:
                W2_sb = ffn.tile([128, 32, D], BF16, tag="w2")
                w2_v = W2Tb[:, :].rearrange("(c p) n -> p c n", p=128)
                b1_sb = ffn.tile([128, 32], F32, tag="b1")
                nc.sync.dma_start(out=b1_sb, in_=b1r[:, :])

                x1_sb = ffn.tile([128, 4, D], BF16, tag="x1")
                x2_sb = ffn.tile([128, 4, D], F32, tag="x2")
                x1T_sb = ffn.tile([128, 8, NT], BF16, tag="x1T")

                # prefetch the first FFN1 weight blocks + first W2 quarter so
                # FFN1 never waits on DMA (covers the old ~1.5us stall)
                with tc.tile_pool(name="w1p", bufs=5) as w1p:
                    w1s = {}
                    for ft in range(5):
                        w1 = w1p.tile([128, 8, 128], BF16, tag="w1col", name="w1")
                        nc.sync.dma_start(
                            out=w1.rearrange("p c n -> p (c n)"), in_=W1S[ft, :, :]
                        )
                        w1s[ft] = w1
                    nc.sync.dma_start(out=W2_sb[:, 0:4, :], in_=w2_v[:, 0:4, :])

                    # --- phase 3: out-proj, residual, LN1, transpose ---
                    with (
                        tc.tile_pool(name="p3", bufs=1) as p3,
                        tc.tile_pool(name="psS2", bufs=2, space="PSUM") as psS2,
                        tc.tile_pool(name="psT", bufs=2, space="PSUM") as psT,
                    ):
                        if trivial_affine:
                            g1bc = be1bc = None
                        else:
                            g1bc = p3.tile([128, D], F32, tag="g1bc")
                            be1bc = p3.tile([128, D], F32, tag="be1bc")
                            for t_, src_ in ((g1bc, g1), (be1bc, be1)):
                                nc.sync.dma_start(
                                    out=t_, in_=src_[:, :].to_broadcast([128, D])
                                )

                        def transposes(nt):
                            for c in range(8):
                                tp = psT.tile([128, 128], BF16, tag="tp", name="tp")
                                nc.tensor.transpose(
                                    tp, x1_sb[:, nt, c * 128 : c * 128 + 128], identb
                                )
                                dst = x1T_sb[:, c, nt * 128 : nt * 128 + 128]
                                if c % 4 != 3:
                                    nc.scalar.activation(out=dst, in_=tp, func=AF.Copy)
                                else:
                                    nc.vector.tensor_copy(out=dst, in_=tp)

                        for nt in range(4):
                            stats = lnp.tile(
                                [128, 2, 6], F32, tag="ln1stats", name="ln1stats"
                            )
                            for dh in range(2):
                                s2 = psS2.tile([128, 512], F32, tag="s2", name="s2")
                                for c in range(8):
                                    nc.tensor.matmul(
                                        s2,
                                        OT_sb[:, c, nt * 128 : nt * 128 + 128],
                                        wof[:, c, dh * 512 : dh * 512 + 512],
                                        start=(c == 0), stop=(c == 7),
                                    )
                                hslc = slice(dh * 512, dh * 512 + 512)
                                nc.vector.tensor_add(
                                    out=x1_sb[:, nt, hslc],
                                    in0=s2,
                                    in1=srar[:, nt, hslc],
                                )
                                nc.vector.bn_stats(
                                    out=stats[:, dh, :], in_=x1_sb[:, nt, hslc]
                                )
                            if nt >= 1:
                                transposes(nt - 1)
                            mv = lnp.tile([128, 2], F32, tag="lnmv", name="lnmv")
                            nc.vector.bn_aggr(out=mv, in_=stats)
                            nc.scalar.activation(
                                out=mv[:, 1:2], in_=mv[:, 1:2], func=AF.Sqrt,
                                bias=eps_sb, scale=1.0,
                            )
                            nc.vector.reciprocal(out=mv[:, 1:2], in_=mv[:, 1:2])
                            if g1bc is None:
                                # ACT applies half 0 (x*rstd - mu*rstd) while
                                # DVE applies half 1 in parallel
                                nmr = lnp.tile([128, 1], F32, tag="nmr1", name="nmr1")
                                nc.vector.tensor_scalar(
                                    out=nmr, in0=mv[:, 0:1],
                                    scalar1=mv[:, 1:2], scalar2=-1.0,
                                    op0=OP.mult, op1=OP.mult,
                                )
                                nc.scalar.activation(
                                    out=x1_sb[:, nt, 0:512], in_=x1_sb[:, nt, 0:512],
                                    func=AF.Identity, bias=nmr, scale=mv[:, 1:2],
                                )
                                nc.vector.tensor_scalar(
                                    out=x1_sb[:, nt, 512:D], in0=x1_sb[:, nt, 512:D],
                                    scalar1=mv[:, 0:1], scalar2=mv[:, 1:2],
                                    op0=OP.subtract, op1=OP.mult,
                                )
                            else:
                                nc.vector.tensor_scalar(
                                    out=x1_sb[:, nt, :], in0=x1_sb[:, nt, :],
                                    scalar1=mv[:, 0:1], scalar2=mv[:, 1:2],
                                    op0=OP.subtract, op1=OP.mult,
                                )
                                nc.vector.tensor_mul(
                                    out=x1_sb[:, nt, :], in0=x1_sb[:, nt, :], in1=g1bc
                                )
                                nc.vector.tensor_add(
                                    out=x1_sb[:, nt, :], in0=x1_sb[:, nt, :], in1=be1bc
                                )
                        transposes(3)

                    # --- phase 4: FFN1 (gelu into bf16 h1T) ---
                    h1T_sb = ffn.tile([128, 32, NT], BF16, tag="h1T")
                    with tc.tile_pool(name="psH", bufs=3, space="PSUM") as psH:
                        for ft in range(32):
                            if ft in w1s:
                                w1 = w1s.pop(ft)
                            else:
                                w1 = w1p.tile(
                                    [128, 8, 128], BF16, tag="w1col", name="w1"
                                )
                                nc.sync.dma_start(
                                    out=w1.rearrange("p c n -> p (c n)"),
                                    in_=W1S[ft, :, :],
                                )
                            if ft % 4 == 0 and ft > 0:
                                q = ft // 4
                                nc.sync.dma_start(
                                    out=W2_sb[:, q * 4 : q * 4 + 4, :],
                                    in_=w2_v[:, q * 4 : q * 4 + 4, :],
                                )
                            if ft < 31:
                                hps = psH.tile([128, NT], F32, tag="h1", name="hps")
                                for dc in range(8):
                                    nc.tensor.matmul(
                                        hps, w1[:, dc, :], x1T_sb[:, dc, :],
                                        start=(dc == 0), stop=(dc == 7),
                                    )
                                nc.scalar.activation(
                                    out=h1T_sb[:, ft, :], in_=hps, func=AF.Gelu,
                                    bias=b1_sb[:, ft : ft + 1], scale=1.0,
                                )
                            else:
                                # split the final column so FFN2 isn't gated
                                # on one full-width trailing gelu
                                for qh in range(2):
                                    qs = slice(qh * 256, qh * 256 + 256)
                                    hps = psH.tile(
                                        [128, 256], F32, tag="h1b", name="hps",
                                        bufs=2,
                                    )
                                    for dc in range(8):
                                        nc.tensor.matmul(
                                            hps, w1[:, dc, :], x1T_sb[:, dc, qs],
                                            start=(dc == 0), stop=(dc == 7),
                                        )
                                    nc.scalar.activation(
                                        out=h1T_sb[:, ft, qs], in_=hps,
                                        func=AF.Gelu,
                                        bias=b1_sb[:, ft : ft + 1], scale=1.0,
                                    )

                # --- phase 5: FFN2 + residual + LN2 + store (bf16) ---
                out_v = out[:, :].rearrange("(nt p) d -> p nt d", p=128)
                obuf = ffn.tile([128, 4, D], BF16, tag="obuf")
                with tc.tile_pool(name="psY", bufs=3, space="PSUM") as psY:
                    if trivial_affine:
                        b2bc = g2bc = be2bc = None
                    else:
                        b2bc = ffn.tile([128, D], F32, tag="b2bc")
                        g2bc = ffn.tile([128, D], F32, tag="g2bc")
                        be2bc = ffn.tile([128, D], F32, tag="be2bc")
                        for t_, src_ in ((b2bc, b2), (g2bc, g2), (be2bc, be2)):
                            nc.sync.dma_start(
                                out=t_, in_=src_[:, :].to_broadcast([128, D])
                            )
                    for nt in range(4):
                        last = nt == 3
                        # finer psum/stat groups on the last chunk shrink the
                        # post-final-matmul critical chain
                        plan = (
                            [(0, 512), (512, 256), (768, 128), (896, 128)]
                            if last
                            else [(0, 512), (512, 512)]
                        )
                        stats = lnp.tile(
                            [128, len(plan), 6], F32,
                            tag=f"ln2stats{len(plan)}", name="ln2stats",
                        )
                        for gi, (c0, cl) in enumerate(plan):
                            yps = psY.tile(
                                [128, cl], F32, tag=f"y{cl}", name="yps",
                                bufs=(3 if cl == 512 else 2 if cl == 128 else 1),
                            )
                            for fc in range(32):
                                nc.tensor.matmul(
                                    yps,
                                    h1T_sb[:, fc, nt * 128 : nt * 128 + 128],
                                    W2_sb[:, fc, c0 : c0 + cl],
                                    start=(fc == 0), stop=(fc == 31),
                                )
                            hslc = slice(c0, c0 + cl)
                            nc.vector.tensor_add(
                                out=x2_sb[:, nt, hslc],
                                in0=yps,
                                in1=x1_sb[:, nt, hslc],
                            )
                            if b2bc is not None:
                                nc.vector.tensor_add(
                                    out=x2_sb[:, nt, hslc],
                                    in0=x2_sb[:, nt, hslc],
                                    in1=b2bc[:, hslc],
                                )
                            nc.vector.bn_stats(
                                out=stats[:, gi, :], in_=x2_sb[:, nt, hslc]
                            )
                        mv = lnp.tile([128, 2], F32, tag="ln2mv", name="ln2mv")
                        nc.vector.bn_aggr(out=mv, in_=stats)
                        nc.scalar.activation(
                            out=mv[:, 1:2], in_=mv[:, 1:2], func=AF.Sqrt,
                            bias=eps_sb, scale=1.0,
                        )
                        nc.vector.reciprocal(out=mv[:, 1:2], in_=mv[:, 1:2])
                        if last:
                            # -mu*rstd so ACT can apply LN as x*rstd + bias
                            nmr = lnp.tile([128, 1], F32, tag="nmr", name="nmr")
                            nc.vector.tensor_scalar(
                                out=nmr, in0=mv[:, 0:1],
                                scalar1=mv[:, 1:2], scalar2=-1.0,
                                op0=OP.mult, op1=OP.mult,
                            )
                            aplan = [(0, 256), (256, 256), (512, 256), (768, 256)]
                        else:
                            nmr = None
                            aplan = [(0, 512), (512, 512)]
                        for ai, (c0, cl) in enumerate(aplan):
                            hslc = slice(c0, c0 + cl)
                            if g2bc is None:
                                if last and ai % 2 == 0:
                                    nc.scalar.activation(
                                        out=obuf[:, nt, hslc],
                                        in_=x2_sb[:, nt, hslc],
                                        func=AF.Identity, bias=nmr,
                                        scale=mv[:, 1:2],
                                    )
                                else:
                                    nc.vector.tensor_scalar(
                                        out=obuf[:, nt, hslc],
                                        in0=x2_sb[:, nt, hslc],
                                        scalar1=mv[:, 0:1], scalar2=mv[:, 1:2],
                                        op0=OP.subtract, op1=OP.mult,
                                    )
                            else:
                                nc.vector.tensor_scalar(
                                    out=x2_sb[:, nt, hslc], in0=x2_sb[:, nt, hslc],
                                    scalar1=mv[:, 0:1], scalar2=mv[:, 1:2],
                                    op0=OP.subtract, op1=OP.mult,
                                )
                                nc.vector.tensor_mul(
                                    out=x1_sb[:, nt, hslc],
                                    in0=x2_sb[:, nt, hslc], in1=g2bc[:, hslc],
                                )
                                nc.vector.tensor_add(
                                    out=obuf[:, nt, hslc],
                                    in0=x2_sb[:, nt, hslc], in1=be2bc[:, hslc],
                                )
                            if not last:
                                nc.sync.dma_start(
                                    out=out_v[:, nt, hslc], in_=obuf[:, nt, hslc]
                                )
                            elif ai == 1:
                                nc.sync.dma_start(
                                    out=out_v[:, nt, 0:512], in_=obuf[:, nt, 0:512]
                                )
                            elif ai == 3:
                                nc.sync.dma_start(
                                    out=out_v[:, nt, 512:D], in_=obuf[:, nt, 512:D]
                                )

    nc.finalize()
    return nc


def host_prep(inputs):
    """Build the 8 per-core input maps from the full problem inputs."""
    src = np.asarray(inputs["src"], np.float32)
    coords = np.asarray(inputs["coords"])
    Wq = np.asarray(inputs["Wq"], np.float32)
    Wk = np.asarray(inputs["Wk"], np.float32)
    Wv = np.asarray(inputs["Wv"], np.float32)
    Wo = np.asarray(inputs["Wo"], np.float32)
    W1 = np.asarray(inputs["W1"], np.float32)
    b1 = np.asarray(inputs["b1"], np.float32)
    W2 = np.asarray(inputs["W2"], np.float32)
    b2 = np.asarray(inputs["b2"], np.float32)
    g1 = np.asarray(inputs["g1"], np.float32)
    be1 = np.asarray(inputs["be1"], np.float32)
    g2 = np.asarray(inputs["g2"], np.float32)
    be2 = np.asarray(inputs["be2"], np.float32)

    # per-head q scaling: scores are computed as S/slope_h (slope re-applied
    # as the exp scale), so Wq columns of head h carry SCALE/slope_h.
    colscale = (SCALE / SLOPES)[np.repeat(np.arange(H), HD)]  # [D]
    WqTs = Wq.T * colscale[None, :]

    def swz(MT):
        # S[bt, p, dc*128+j] = MT[dc*128+p, bt*128+j]
        return np.ascontiguousarray(
            MT.reshape(8, 128, 8, 128).transpose(2, 1, 0, 3).reshape(8, 128, D)
        ).astype(BF)

    shared = {
        "WqS": swz(WqTs),
        "WkS": swz(Wk.T),
        "WvS": swz(Wv.T),
        "WoT": np.ascontiguousarray(Wo.T).astype(BF),
        # W1S[ft, p, dc*128+j] = W1.T[dc*128+p, ft*128+j]
        "W1S": np.ascontiguousarray(
            W1.T.reshape(8, 128, 32, 128).transpose(2, 1, 0, 3).reshape(32, 128, D)
        ).astype(BF),
        "W2Tb": np.ascontiguousarray(W2.T).astype(BF),
        "b1r": np.ascontiguousarray(b1.reshape(32, 128).T),
        "b2": b2.reshape(1, D),
        "g1": g1.reshape(1, D),
        "be1": be1.reshape(1, D),
        "g2": g2.reshape(1, D),
        "be2": be2.reshape(1, D),
    }

    in_maps = []
    for c in range(NCORES):
        b = c // 2
        half = c % 2
        rows = slice(half * NT, (half + 1) * NT)
        # token permutation: own query half first (key/V order is arbitrary
        # as long as srcT and kaug_x agree)
        idx = np.r_[half * NT : (half + 1) * NT, (1 - half) * NT : (2 - half) * NT]
        x = coords[b, :, 0].astype(np.float64)
        y = coords[b, :, 1].astype(np.float64)
        s = (x + y).astype(np.float32)
        thr = np.arange(1, GRID, dtype=np.float64)
        cx = (x[None, :] >= thr[:, None]).astype(np.float32)
        cy = (y[None, :] >= thr[:, None]).astype(np.float32)
        kaug = np.concatenate(
            [s.reshape(1, N), np.zeros((1, N), np.float32), cx, cy], axis=0
        )[:, idx].astype(BF)
        qaug = np.empty((H, 64, NT), np.float32)
        qaug[:, 0, :] = 1.0
        qaug[:, 1, :] = 0.0
        qaug[:, 2:33, :] = -2.0 * cx[None, :, rows]
        qaug[:, 33:64, :] = -2.0 * cy[None, :, rows]
        srcTb = src[b].T
        m = dict(shared)
        m.update(
            {
                "srcT": np.ascontiguousarray(srcTb[:, idx]).astype(BF),
                "src_rows": np.ascontiguousarray(src[b, rows, :]),
                "kaug_x": kaug,
                "qaug_x": qaug.astype(BF),
            }
        )
        in_maps.append(m)
    return in_maps


_NCS = {}
LAST_RUN_S = None


def get_nc(trivial_affine=True):
    if trivial_affine not in _NCS:
        _NCS[trivial_affine] = build_nc(trivial_affine)
    return _NCS[trivial_affine]


def _affine_trivial(inputs):
    return (
        np.all(np.asarray(inputs["g1"]) == 1.0)
        and np.all(np.asarray(inputs["g2"]) == 1.0)
        and not np.any(np.asarray(inputs["be1"]))
        and not np.any(np.asarray(inputs["be2"]))
        and not np.any(np.asarray(inputs["b2"]))
    )


def kernel(**inputs):
    global LAST_RUN_S
    from concourse.bass_utils import run_bass_kernel_spmd

    nc = get_nc(bool(_affine_trivial(inputs)))
    in_maps = host_prep(inputs)
    t0 = time.monotonic()
    res = run_bass_kernel_spmd(nc, in_maps, list(range(NCORES)))
    LAST_RUN_S = time.monotonic() - t0
    full = np.empty((B, N, D), np.float32)
    for c in range(NCORES):
        b = c // 2
        half = c % 2
        full[b, half * NT : (half + 1) * NT, :] = np.asarray(
            res.results[c]["out"]
        ).astype(np.float32)
    return full


# revision 3
# speedup vs baseline: 1.0342x; 1.0342x over previous
"""Fused transformer encoder layer (attention w/ 2D-ALiBi bias + FFN) on 8 trn2 cores.

Sharding: core c handles batch b = c//2, token half h = c%2 (512 query rows).
K/V are computed per-core for the full 1024-token sequence of its batch
(duplicated across the 2 cores sharing a batch); outputs are disjoint row
slices of the final tensor, so no collectives are needed.

Bias trick: the alibi_2d bias slope_h*(|xi-xj|+|yi-yj|) is folded into the
QK^T contraction. |xi-xj| = xi + xj - 2*a_i.a_j with a_i in {0,1}^31 the
threshold indicators of xi, so dist(i,j) = s_i + s_j - 2*c_i.c_j (c = 62-dim
indicator, s = x+y). The per-query term slope*s_i is constant along the
softmax axis and is dropped. Q/K are augmented with 64 extra contraction dims
(s_j / pad / c_j on the K side; 1 / 0 / -2*c_i on the Q side), making the
score contraction K = 64+64 = 128 exactly — full PE array, bias for free.

bf16 precision care: the aug rows are small integers / {0,-2} — exact in
bf16. The attention scale AND the per-head slope are folded out of the bf16
data: Q-projection weights carry scale/slope_h per head (so scores come out
as S/slope_h) and the exact fp32 slope_h is re-applied as the exp()
activation's scale immediate. exp needs no max-subtraction (|S| <= ~50 by
construction).

v2 attention dataflow: scores are computed keys-on-partitions (S^T) so exp()
output P^T feeds the AV matmuls as the STATIONARY operand (chunks [128k,128q])
with V as the 65-wide moving operand (64 dims + a ones column that yields the
softmax denominator). This produces O in [query, head-dim] layout at 65 moving
cols per (head, q-chunk, key-chunk) instead of 512 — ~2x less PE time in AV —
and makes the softmax normalization a plain per-partition (per-query)
tensor_scalar multiply. Normalized O tiles are PE-transposed back to O^T for
the head-summing out-projection. The V projection is computed per head-pair
and interleaved into the attention head loop, filling PE slack under the
ACT-bound exp stream.
"""

import math
import sys
import time

for _p in ("/opt/trn_rl_repo",):
    if _p not in sys.path:
        sys.path.insert(0, _p)

import numpy as np
import ml_dtypes

import concourse.bass as bass
import concourse.tile as tile
from concourse import bacc, mybir
from concourse.masks import make_identity

F32 = mybir.dt.float32
BF16 = mybir.dt.bfloat16
BF = ml_dtypes.bfloat16

D = 1024          # d_model
H = 16            # heads
HD = 64           # head dim
DFF = 4096
B = 4
N = 1024          # sequence length
NT = 512          # tokens (query rows) per core
GRID = 32
EPS = 1e-5
NCORES = 8
SCALE = HD ** -0.5


def _alibi_slopes(n):
    def pow2(n_):
        start = 2.0 ** (-(2.0 ** -(math.log2(n_) - 3)))
        return [start * start ** i for i in range(n_)]
    if math.log2(n).is_integer():
        return np.array(pow2(n), dtype=np.float64)
    m = 2 ** math.floor(math.log2(n))
    s = pow2(m)
    s += [s[-1] * 0.5 ** (i + 1) for i in range(n - m)]
    return np.array(s, dtype=np.float64)


SLOPES = _alibi_slopes(H)


def build_nc(trivial_affine=False):
    """trivial_affine: g1/g2 all-ones and be1/be2/b2 all-zeros -> skip those ops."""
    nc = bacc.Bacc()

    # srcT columns are host-permuted per core: own query half first, so the
    # Q projection reads stf[:, :, 0:512] (keys/V follow the same order —
    # softmax is order-invariant as long as kaug_x matches).
    srcT = nc.declare_dram_parameter("srcT", [D, N], BF16, isOutput=False)
    src_rows = nc.declare_dram_parameter("src_rows", [NT, D], F32, isOutput=False)
    # W{q,k,v}S[bt, p, dc*128+j] = W*T[dc*128+p, bt*128+j]: each 128-wide
    # output block is one fully-contiguous 256KB DMA (2KB runs/partition).
    WqS = nc.declare_dram_parameter("WqS", [8, 128, D], BF16, isOutput=False)
    WkS = nc.declare_dram_parameter("WkS", [8, 128, D], BF16, isOutput=False)
    WvS = nc.declare_dram_parameter("WvS", [8, 128, D], BF16, isOutput=False)
    WoT = nc.declare_dram_parameter("WoT", [D, D], BF16, isOutput=False)
    # W1 pre-swizzled on host: W1S[ft, p, dc*128+j] = W1.T[dc*128+p, ft*128+j]
    # so each FFN1 weight block is one fully-contiguous 256KB DMA.
    W1S = nc.declare_dram_parameter("W1S", [32, 128, D], BF16, isOutput=False)
    W2Tb = nc.declare_dram_parameter("W2Tb", [DFF, D], BF16, isOutput=False)
    kaug_x = nc.declare_dram_parameter("kaug_x", [64, N], BF16, isOutput=False)
    qaug_x = nc.declare_dram_parameter("qaug_x", [H, 64, NT], BF16, isOutput=False)
    b1r = nc.declare_dram_parameter("b1r", [128, 32], F32, isOutput=False)
    b2 = nc.declare_dram_parameter("b2", [1, D], F32, isOutput=False)
    g1 = nc.declare_dram_parameter("g1", [1, D], F32, isOutput=False)
    be1 = nc.declare_dram_parameter("be1", [1, D], F32, isOutput=False)
    g2 = nc.declare_dram_parameter("g2", [1, D], F32, isOutput=False)
    be2 = nc.declare_dram_parameter("be2", [1, D], F32, isOutput=False)
    out = nc.declare_dram_parameter("out", [NT, D], BF16, isOutput=True)

    AF = mybir.ActivationFunctionType
    OP = mybir.AluOpType

    with tile.TileContext(nc) as tc:
        with (
            tc.tile_pool(name="misc", bufs=1) as misc,
            tc.tile_pool(name="lnp", bufs=4) as lnp,
        ):
            eps_sb = misc.tile([128, 1], F32, tag="eps")
            nc.vector.memset(eps_sb, EPS)
            ident = misc.tile([128, 128], F32, tag="ident")
            make_identity(nc, ident)
            identb = misc.tile([128, 128], BF16, tag="identb")
            make_identity(nc, identb)
            # OT_sb[p, c, q]: head 2c in partitions 0:64, head 2c+1 in 64:128
            OT_sb = misc.tile([128, 8, NT], BF16, tag="otsb")
            # wof/srar live in the outer scope so their DMAs don't alias
            # (and thus wait on) attention SBUF: issued mid-attention, ready
            # well before the out-projection needs them.
            wof = misc.tile([128, 8, D], BF16, tag="wof")
            srar = misc.tile([128, 4, D], F32, tag="srcrows")

            def ln_apply(x_ap, gbc, bbc):
                stats = lnp.tile([128, 2, 6], F32, tag="lnstats", name="lnstats")
                for sg in range(2):
                    nc.vector.bn_stats(
                        out=stats[:, sg, :], in_=x_ap[:, sg * 512 : sg * 512 + 512]
                    )
                mv = lnp.tile([128, 2], F32, tag="lnmv", name="lnmv")
                nc.vector.bn_aggr(out=mv, in_=stats)
                nc.scalar.activation(
                    out=mv[:, 1:2], in_=mv[:, 1:2], func=AF.Sqrt,
                    bias=eps_sb, scale=1.0,
                )
                nc.vector.reciprocal(out=mv[:, 1:2], in_=mv[:, 1:2])
                nc.vector.tensor_scalar(
                    out=x_ap, in0=x_ap,
                    scalar1=mv[:, 0:1], scalar2=mv[:, 1:2],
                    op0=OP.subtract, op1=OP.mult,
                )
                if gbc is not None:
                    nc.vector.tensor_mul(out=x_ap, in0=x_ap, in1=gbc)
                if bbc is not None:
                    nc.vector.tensor_add(out=x_ap, in0=x_ap, in1=bbc)

            # ============ attention scope ============
            with tc.tile_pool(name="att", bufs=1) as att:
                kaug = att.tile([128, H, N], BF16, tag="kaug")
                qaug = att.tile([128, H, NT], BF16, tag="qaug")
                # v_sb[p, mt, h, j]: V[token mt*128+p, head h dim j]; j=64 is
                # the ones column (softmax denominator via AV contraction).
                v_sb = att.tile([128, 8, H, 65], BF16, tag="vsb")
                nc.vector.memset(v_sb[:, :, :, 64], 1.0)
                # stf/wvf live for the whole attention scope: the V projection
                # for pairs 1-7 is interleaved into the head loop below.
                stf = att.tile([128, 8, N], BF16, tag="stf")
                wvf = att.tile([128, 8, D], BF16, tag="wvf")

                # --- phase 1: Q/K projections + V proj for pair 0 ---
                with tc.tile_pool(name="ph1", bufs=1) as ph1:
                    # fine-grained loads in consumption order so the PE never
                    # outruns the (serialized) DMA stream by much
                    st_vw = srcT[:, :].rearrange("(c p) n -> p c n", p=128)
                    wqf = ph1.tile([128, 8, D], BF16, tag="wqf")
                    wkf = ph1.tile([128, 8, D], BF16, tag="wkf")
                    nc.sync.dma_start(out=stf[:, 0, 0:NT], in_=st_vw[:, 0, 0:NT])
                    nc.sync.dma_start(out=wqf[:, 0, 0:128], in_=WqS[0, :, 0:128])
                    nc.sync.dma_start(out=stf[:, 1, 0:NT], in_=st_vw[:, 1, 0:NT])
                    nc.sync.dma_start(out=wqf[:, 0, 128:D], in_=WqS[0, :, 128:D])
                    nc.sync.dma_start(out=stf[:, 2, 0:NT], in_=st_vw[:, 2, 0:NT])
                    nc.sync.dma_start(out=stf[:, 3, 0:NT], in_=st_vw[:, 3, 0:NT])
                    nc.sync.dma_start(out=stf[:, 4:6, 0:NT], in_=st_vw[:, 4:6, 0:NT])
                    nc.sync.dma_start(out=stf[:, 6:8, 0:NT], in_=st_vw[:, 6:8, 0:NT])
                    for dt in range(1, 8):
                        nc.sync.dma_start(out=wqf[:, dt, :], in_=WqS[dt, :, :])
                    nc.sync.dma_start(out=wkf[:, 0, :], in_=WkS[0, :, :])
                    nc.sync.dma_start(out=stf[:, 0:4, NT:N], in_=st_vw[:, 0:4, NT:N])
                    nc.sync.dma_start(out=stf[:, 4:8, NT:N], in_=st_vw[:, 4:8, NT:N])
                    for dt in range(1, 8):
                        nc.sync.dma_start(out=wkf[:, dt, :], in_=WkS[dt, :, :])
                    for p in range(8):
                        nc.sync.dma_start(out=wvf[:, p, :], in_=WvS[p, :, :])
                    for h in range(H):
                        nc.sync.dma_start(out=kaug[64:128, h, :], in_=kaug_x[:, :])
                        nc.sync.dma_start(out=qaug[64:128, h, :], in_=qaug_x[h, :, :])
                    # early prefetch for phase 3 (DMA engines idle mid-attention)
                    nc.sync.dma_start(
                        out=wof, in_=WoT[:, :].rearrange("(c p) n -> p c n", p=128)
                    )
                    nc.sync.dma_start(
                        out=srar,
                        in_=src_rows[:, :].rearrange("(nt p) d -> p nt d", p=128),
                    )

                    # Q + K projections -> qaug/kaug top halves (shared pool)
                    with tc.tile_pool(name="psQK", bufs=3, space="PSUM") as psQK:
                        for dt in range(8):
                            qps = psQK.tile([128, NT], F32, tag="proj", name="qps")
                            for dc in range(8):
                                nc.tensor.matmul(
                                    qps,
                                    wqf[:, dt, dc * 128 : dc * 128 + 128],
                                    stf[:, dc, 0:NT],
                                    start=(dc == 0), stop=(dc == 7),
                                )
                            nc.scalar.activation(
                                out=qaug[0:64, 2 * dt, :], in_=qps[0:64, :], func=AF.Copy
                            )
                            nc.vector.tensor_copy(
                                out=qaug[0:64, 2 * dt + 1, :], in_=qps[64:128, :]
                            )
                        for dt in range(8):
                            for mh in range(2):
                                kps = psQK.tile([128, 512], F32, tag="proj", name="kps")
                                for dc in range(8):
                                    nc.tensor.matmul(
                                        kps,
                                        wkf[:, dt, dc * 128 : dc * 128 + 128],
                                        stf[:, dc, mh * 512 : mh * 512 + 512],
                                        start=(dc == 0), stop=(dc == 7),
                                    )
                                nc.scalar.activation(
                                    out=kaug[0:64, 2 * dt, mh * 512 : mh * 512 + 512],
                                    in_=kps[0:64, :], func=AF.Copy,
                                )
                                nc.vector.tensor_copy(
                                    out=kaug[0:64, 2 * dt + 1, mh * 512 : mh * 512 + 512],
                                    in_=kps[64:128, :],
                                )
                        # V projection for pair 0 only (pairs 1-7 are
                        # interleaved into the attention head loop below)
                        for mt in range(8):
                            vp = psQK.tile([128, 128], F32, tag="vp1", name="vp")
                            for dc in range(8):
                                nc.tensor.matmul(
                                    vp,
                                    stf[:, dc, mt * 128 : mt * 128 + 128],
                                    wvf[:, 0, dc * 128 : dc * 128 + 128],
                                    start=(dc == 0), stop=(dc == 7),
                                )
                            vdst = v_sb[:, mt, 0:2, 0:64]
                            vsrc = vp.rearrange("p (h w) -> p h w", w=64)
                            if mt % 2 == 0:
                                nc.scalar.activation(out=vdst, in_=vsrc, func=AF.Copy)
                            else:
                                nc.vector.tensor_copy(out=vdst, in_=vsrc)

                # --- phase 2: attention head loop ---
                # step s: scores+exp(h=s) | AV+normalize(h=s-1) | transpose
                # (pair s//2-1 at even s) | V proj half-unit for pair s//2+1.
                with (
                    tc.tile_pool(name="ptp", bufs=4) as ptp,
                    tc.tile_pool(name="osbp", bufs=2) as osbp,
                    tc.tile_pool(name="recp", bufs=2) as recp,
                    tc.tile_pool(name="psS", bufs=2, space="PSUM") as psS,
                    tc.tile_pool(name="psAV", bufs=2, space="PSUM") as psAV,
                    tc.tile_pool(name="psTP", bufs=1, space="PSUM") as psTP,
                    tc.tile_pool(name="psV2", bufs=1, space="PSUM") as psV2,
                ):
                    pts = {}
                    avps = {}
                    osbs = {}

                    def v_chunk(p, mt):
                        vp = psV2.tile([128, 128], F32, tag="vp2", name="vp2")
                        for dc in range(8):
                            nc.tensor.matmul(
                                vp,
                                stf[:, dc, mt * 128 : mt * 128 + 128],
                                wvf[:, p, dc * 128 : dc * 128 + 128],
                                start=(dc == 0), stop=(dc == 7),
                            )
                        nc.vector.tensor_copy(
                            out=v_sb[:, mt, 2 * p : 2 * p + 2, 0:64],
                            in_=vp.rearrange("p (h w) -> p h w", w=64),
                        )

                    # AV(h) runs at step h+2 (not h+1): a full extra step of
                    # slack for the ACT exp stream to finish pt(h).
                    for step in range(H + 2):
                        h = step if step < H else None
                        hp = step - 2 if step >= 2 else None
                        # V-proj half-unit for pair step//2+1 (mts by parity)
                        vjobs = []
                        if step < 14:
                            pv = step // 2 + 1
                            vjobs = [(pv, (step % 2) * 4 + j) for j in range(4)]

                        if h is not None:
                            pt = ptp.tile([128, 8, NT], BF16, tag="pt", name="pt")
                            pts[h] = pt
                        if hp is not None:
                            avp = psAV.tile(
                                [128, 4, 128], F32, tag="avp", name="avp"
                            )
                            avps[hp] = avp

                        # scores g0/g1 + first half of AV(hp) + V work,
                        # then scores g2/g3, AV second half, transposes.
                        for g in range(4):
                            if h is not None:
                                stg = psS.tile(
                                    [128, 2, NT], F32, tag="stg", name="stg"
                                )
                                for j in range(2):
                                    mt = 2 * g + j
                                    nc.tensor.matmul(
                                        stg[:, j, :],
                                        kaug[:, h, mt * 128 : mt * 128 + 128],
                                        qaug[:, h, :],
                                        start=True, stop=True,
                                    )
                                nc.scalar.activation(
                                    out=pt[:, 2 * g : 2 * g + 2, :], in_=stg,
                                    func=AF.Exp, scale=float(SLOPES[h]),
                                )
                            if vjobs and g % 2 == 1:
                                for p, mt in vjobs[2 * (g // 2) : 2 * (g // 2) + 2]:
                                    v_chunk(p, mt)
                            if hp is not None:
                                ptc = pts[hp]
                                for qc in (g,):
                                    for mt in range(8):
                                        nc.tensor.matmul(
                                            avp[:, qc, 0:65],
                                            ptc[:, mt, qc * 128 : qc * 128 + 128],
                                            v_sb[:, mt, hp, :],
                                            start=(mt == 0), stop=(mt == 7),
                                        )

                        if hp is not None:
                            pts.pop(hp)
                            avp = avps.pop(hp)
                            c = hp // 2
                            if hp % 2 == 0:
                                osb = osbp.tile(
                                    [128, 4, 128], BF16, tag="osb", name="osb"
                                )
                                osbs[c] = osb
                            osb = osbs[c]
                            hcol = (hp % 2) * 64
                            rec = recp.tile([128, 4, 1], F32, tag="rec", name="rec")
                            nc.vector.reciprocal(out=rec, in_=avp[:, :, 64:65])
                            for qc in range(4):
                                nc.vector.tensor_scalar(
                                    out=osb[:, qc, hcol : hcol + 64],
                                    in0=avp[:, qc, 0:64],
                                    scalar1=rec[:, qc, :], scalar2=None,
                                    op0=OP.mult,
                                )
                            if hp % 2 == 1:
                                osb = osbs.pop(c)
                                for qc in range(4):
                                    tp = psTP.tile(
                                        [128, 128], BF16, tag="tp", name="tp"
                                    )
                                    nc.tensor.transpose(tp, osb[:, qc, :], identb)
                                    nc.vector.tensor_copy(
                                        out=OT_sb[:, c, qc * 128 : qc * 128 + 128],
                                        in_=tp,
                                    )

            # ============ post-attention scope ============
            with tc.tile_pool(name="ffn", bufs=1) as ffn:
                W2_sb = ffn.tile([128, 32, D], BF16, tag="w2")
                w2_v = W2Tb[:, :].rearrange("(c p) n -> p c n", p=128)
                b1_sb = ffn.tile([128, 32], F32, tag="b1")
                nc.sync.dma_start(out=b1_sb, in_=b1r[:, :])

                x1_sb = ffn.tile([128, 4, D], BF16, tag="x1")
                x2_sb = ffn.tile([128, 4, D], F32, tag="x2")
                x1T_sb = ffn.tile([128, 8, NT], BF16, tag="x1T")

                # prefetch the first FFN1 weight blocks + first W2 quarter so
                # FFN1 never waits on DMA (covers the old ~1.5us stall)
                with tc.tile_pool(name="w1p", bufs=7) as w1p:
                    w1s = {}
                    for ft in range(7):
                        w1 = w1p.tile([128, 8, 128], BF16, tag="w1col", name="w1")
                        nc.sync.dma_start(
                            out=w1.rearrange("p c n -> p (c n)"), in_=W1S[ft, :, :]
                        )
                        w1s[ft] = w1
                    nc.sync.dma_start(out=W2_sb[:, 0:4, :], in_=w2_v[:, 0:4, :])
                    nc.sync.dma_start(out=W2_sb[:, 4:8, :], in_=w2_v[:, 4:8, :])

                    # --- phase 3: out-proj, residual, LN1, transpose ---
                    with (
                        tc.tile_pool(name="p3", bufs=1) as p3,
                        tc.tile_pool(name="psS2", bufs=2, space="PSUM") as psS2,
                        tc.tile_pool(name="psT", bufs=2, space="PSUM") as psT,
                    ):
                        if trivial_affine:
                            g1bc = be1bc = None
                        else:
                            g1bc = p3.tile([128, D], F32, tag="g1bc")
                            be1bc = p3.tile([128, D], F32, tag="be1bc")
                            for t_, src_ in ((g1bc, g1), (be1bc, be1)):
                                nc.sync.dma_start(
                                    out=t_, in_=src_[:, :].to_broadcast([128, D])
                                )

                        def transposes(nt):
                            for c in range(8):
                                tp = psT.tile([128, 128], BF16, tag="tp", name="tp")
                                nc.tensor.transpose(
                                    tp, x1_sb[:, nt, c * 128 : c * 128 + 128], identb
                                )
                                dst = x1T_sb[:, c, nt * 128 : nt * 128 + 128]
                                if c % 4 != 3:
                                    nc.scalar.activation(out=dst, in_=tp, func=AF.Copy)
                                else:
                                    nc.vector.tensor_copy(out=dst, in_=tp)

                        for nt in range(4):
                            stats = lnp.tile(
                                [128, 2, 6], F32, tag="ln1stats", name="ln1stats"
                            )
                            for dh in range(2):
                                s2 = psS2.tile([128, 512], F32, tag="s2", name="s2")
                                for c in range(8):
                                    nc.tensor.matmul(
                                        s2,
                                        OT_sb[:, c, nt * 128 : nt * 128 + 128],
                                        wof[:, c, dh * 512 : dh * 512 + 512],
                                        start=(c == 0), stop=(c == 7),
                                    )
                                hslc = slice(dh * 512, dh * 512 + 512)
                                nc.vector.tensor_add(
                                    out=x1_sb[:, nt, hslc],
                                    in0=s2,
                                    in1=srar[:, nt, hslc],
                                )
                                nc.vector.bn_stats(
                                    out=stats[:, dh, :], in_=x1_sb[:, nt, hslc]
                                )
                            if nt >= 1:
                                transposes(nt - 1)
                            mv = lnp.tile([128, 2], F32, tag="lnmv", name="lnmv")
                            nc.vector.bn_aggr(out=mv, in_=stats)
                            nc.scalar.activation(
                                out=mv[:, 1:2], in_=mv[:, 1:2], func=AF.Sqrt,
                                bias=eps_sb, scale=1.0,
                            )
                            nc.vector.reciprocal(out=mv[:, 1:2], in_=mv[:, 1:2])
                            if g1bc is None:
                                # ACT applies half 0 (x*rstd - mu*rstd) while
                                # DVE applies half 1 in parallel
                                nmr = lnp.tile([128, 1], F32, tag="nmr1", name="nmr1")
                                nc.vector.tensor_scalar(
                                    out=nmr, in0=mv[:, 0:1],
                                    scalar1=mv[:, 1:2], scalar2=-1.0,
                                    op0=OP.mult, op1=OP.mult,
                                )
                                nc.scalar.activation(
                                    out=x1_sb[:, nt, 0:512], in_=x1_sb[:, nt, 0:512],
                                    func=AF.Identity, bias=nmr, scale=mv[:, 1:2],
                                )
                                nc.vector.tensor_scalar(
                                    out=x1_sb[:, nt, 512:D], in0=x1_sb[:, nt, 512:D],
                                    scalar1=mv[:, 0:1], scalar2=mv[:, 1:2],
                                    op0=OP.subtract, op1=OP.mult,
                                )
                            else:
                                nc.vector.tensor_scalar(
                                    out=x1_sb[:, nt, :], in0=x1_sb[:, nt, :],
                                    scalar1=mv[:, 0:1], scalar2=mv[:, 1:2],
                                    op0=OP.subtract, op1=OP.mult,
                                )
                                nc.vector.tensor_mul(
                                    out=x1_sb[:, nt, :], in0=x1_sb[:, nt, :], in1=g1bc
                                )
                                nc.vector.tensor_add(
                                    out=x1_sb[:, nt, :], in0=x1_sb[:, nt, :], in1=be1bc
                                )
                        transposes(3)

                    # --- phase 4: FFN1 (gelu into bf16 h1T) ---
                    h1T_sb = ffn.tile([128, 32, NT], BF16, tag="h1T")
                    with tc.tile_pool(name="psH", bufs=3, space="PSUM") as psH:
                        for ft in range(32):
                            if ft in w1s:
                                w1 = w1s.pop(ft)
                            else:
                                w1 = w1p.tile(
                                    [128, 8, 128], BF16, tag="w1col", name="w1"
                                )
                                nc.sync.dma_start(
                                    out=w1.rearrange("p c n -> p (c n)"),
                                    in_=W1S[ft, :, :],
                                )
                            if ft % 4 == 0 and ft >= 8:
                                q = ft // 4
                                nc.sync.dma_start(
                                    out=W2_sb[:, q * 4 : q * 4 + 4, :],
                                    in_=w2_v[:, q * 4 : q * 4 + 4, :],
                                )
                            if ft < 31:
                                hps = psH.tile([128, NT], F32, tag="h1", name="hps")
                                for dc in range(8):
                                    nc.tensor.matmul(
                                        hps, w1[:, dc, :], x1T_sb[:, dc, :],
                                        start=(dc == 0), stop=(dc == 7),
                                    )
                                nc.scalar.activation(
                                    out=h1T_sb[:, ft, :], in_=hps, func=AF.Gelu,
                                    bias=b1_sb[:, ft : ft + 1], scale=1.0,
                                )
                            else:
                                # split the final column so FFN2 isn't gated
                                # on one full-width trailing gelu
                                for qh in range(2):
                                    qs = slice(qh * 256, qh * 256 + 256)
                                    hps = psH.tile(
                                        [128, 256], F32, tag="h1b", name="hps",
                                        bufs=2,
                                    )
                                    for dc in range(8):
                                        nc.tensor.matmul(
                                            hps, w1[:, dc, :], x1T_sb[:, dc, qs],
                                            start=(dc == 0), stop=(dc == 7),
                                        )
                                    nc.scalar.activation(
                                        out=h1T_sb[:, ft, qs], in_=hps,
                                        func=AF.Gelu,
                                        bias=b1_sb[:, ft : ft + 1], scale=1.0,
                                    )

                # --- phase 5: FFN2 + residual + LN2 + store (bf16) ---
                out_v = out[:, :].rearrange("(nt p) d -> p nt d", p=128)
                obuf = ffn.tile([128, 4, D], BF16, tag="obuf")
                with tc.tile_pool(name="psY", bufs=3, space="PSUM") as psY:
                    if trivial_affine:
                        b2bc = g2bc = be2bc = None
                    else:
                        b2bc = ffn.tile([128, D], F32, tag="b2bc")
                        g2bc = ffn.tile([128, D], F32, tag="g2bc")
                        be2bc = ffn.tile([128, D], F32, tag="be2bc")
                        for t_, src_ in ((b2bc, b2), (g2bc, g2), (be2bc, be2)):
                            nc.sync.dma_start(
                                out=t_, in_=src_[:, :].to_broadcast([128, D])
                            )
                    for nt in range(4):
                        last = nt == 3
                        # finer psum/stat groups on the last chunk shrink the
                        # post-final-matmul critical chain
                        plan = (
                            [(0, 512), (512, 256), (768, 128), (896, 64), (960, 64)]
                            if last
                            else [(0, 512), (512, 512)]
                        )
                        stats = lnp.tile(
                            [128, len(plan), 6], F32,
                            tag=f"ln2stats{len(plan)}", name="ln2stats",
                        )
                        for gi, (c0, cl) in enumerate(plan):
                            yps = psY.tile(
                                [128, cl], F32, tag=f"y{cl}", name="yps",
                                bufs=(3 if cl == 512 else 2 if cl == 64 else 1),
                            )
                            for fc in range(32):
                                nc.tensor.matmul(
                                    yps,
                                    h1T_sb[:, fc, nt * 128 : nt * 128 + 128],
                                    W2_sb[:, fc, c0 : c0 + cl],
                                    start=(fc == 0), stop=(fc == 31),
                                )
                            hslc = slice(c0, c0 + cl)
                            nc.vector.tensor_add(
                                out=x2_sb[:, nt, hslc],
                                in0=yps,
                                in1=x1_sb[:, nt, hslc],
                            )
                            if b2bc is not None:
                                nc.vector.tensor_add(
                                    out=x2_sb[:, nt, hslc],
                                    in0=x2_sb[:, nt, hslc],
                                    in1=b2bc[:, hslc],
                                )
                            nc.vector.bn_stats(
                                out=stats[:, gi, :], in_=x2_sb[:, nt, hslc]
                            )
                        mv = lnp.tile([128, 2], F32, tag="ln2mv", name="ln2mv")
                        nc.vector.bn_aggr(out=mv, in_=stats)
                        nc.scalar.activation(
                            out=mv[:, 1:2], in_=mv[:, 1:2], func=AF.Sqrt,
                            bias=eps_sb, scale=1.0,
                        )
                        nc.vector.reciprocal(out=mv[:, 1:2], in_=mv[:, 1:2])
                        if last:
                            # -mu*rstd so ACT can apply LN as x*rstd + bias
                            nmr = lnp.tile([128, 1], F32, tag="nmr", name="nmr")
                            nc.vector.tensor_scalar(
                                out=nmr, in0=mv[:, 0:1],
                                scalar1=mv[:, 1:2], scalar2=-1.0,
                                op0=OP.mult, op1=OP.mult,
                            )
                            aplan = [(0, 256), (256, 256), (512, 256), (768, 256)]
                        else:
                            nmr = None
                            aplan = [(0, 512), (512, 512)]
                        for ai, (c0, cl) in enumerate(aplan):
                            hslc = slice(c0, c0 + cl)
                            if g2bc is None:
                                if last and ai % 2 == 0:
                                    nc.scalar.activation(
                                        out=obuf[:, nt, hslc],
                                        in_=x2_sb[:, nt, hslc],
                                        func=AF.Identity, bias=nmr,
                                        scale=mv[:, 1:2],
                                    )
                                else:
                                    nc.vector.tensor_scalar(
                                        out=obuf[:, nt, hslc],
                                        in0=x2_sb[:, nt, hslc],
                                        scalar1=mv[:, 0:1], scalar2=mv[:, 1:2],
                                        op0=OP.subtract, op1=OP.mult,
                                    )
                            else:
                                nc.vector.tensor_scalar(
                                    out=x2_sb[:, nt, hslc], in0=x2_sb[:, nt, hslc],
                                    scalar1=mv[:, 0:1], scalar2=mv[:, 1:2],
                                    op0=OP.subtract, op1=OP.mult,
                                )
                                nc.vector.tensor_mul(
                                    out=x1_sb[:, nt, hslc],
                                    in0=x2_sb[:, nt, hslc], in1=g2bc[:, hslc],
                                )
                                nc.vector.tensor_add(
                                    out=obuf[:, nt, hslc],
                                    in0=x2_sb[:, nt, hslc], in1=be2bc[:, hslc],
                                )
                            if not last:
                                nc.sync.dma_start(
                                    out=out_v[:, nt, hslc], in_=obuf[:, nt, hslc]
                                )
                            elif ai == 1:
                                nc.sync.dma_start(
                                    out=out_v[:, nt, 0:512], in_=obuf[:, nt, 0:512]
                                )
                            elif ai == 3:
                                nc.sync.dma_start(
                                    out=out_v[:, nt, 512:D], in_=obuf[:, nt, 512:D]
                                )

    nc.finalize()
    return nc


def host_prep(inputs):
    """Build the 8 per-core input maps from the full problem inputs."""
    src = np.asarray(inputs["src"], np.float32)
    coords = np.asarray(inputs["coords"])
    Wq = np.asarray(inputs["Wq"], np.float32)
    Wk = np.asarray(inputs["Wk"], np.float32)
    Wv = np.asarray(inputs["Wv"], np.float32)
    Wo = np.asarray(inputs["Wo"], np.float32)
    W1 = np.asarray(inputs["W1"], np.float32)
    b1 = np.asarray(inputs["b1"], np.float32)
    W2 = np.asarray(inputs["W2"], np.float32)
    b2 = np.asarray(inputs["b2"], np.float32)
    g1 = np.asarray(inputs["g1"], np.float32)
    be1 = np.asarray(inputs["be1"], np.float32)
    g2 = np.asarray(inputs["g2"], np.float32)
    be2 = np.asarray(inputs["be2"], np.float32)

    # per-head q scaling: scores are computed as S/slope_h (slope re-applied
    # as the exp scale), so Wq columns of head h carry SCALE/slope_h.
    colscale = (SCALE / SLOPES)[np.repeat(np.arange(H), HD)]  # [D]
    WqTs = Wq.T * colscale[None, :]

    def swz(MT):
        # S[bt, p, dc*128+j] = MT[dc*128+p, bt*128+j]
        return np.ascontiguousarray(
            MT.reshape(8, 128, 8, 128).transpose(2, 1, 0, 3).reshape(8, 128, D)
        ).astype(BF)

    shared = {
        "WqS": swz(WqTs),
        "WkS": swz(Wk.T),
        "WvS": swz(Wv.T),
        "WoT": np.ascontiguousarray(Wo.T).astype(BF),
        # W1S[ft, p, dc*128+j] = W1.T[dc*128+p, ft*128+j]
        "W1S": np.ascontiguousarray(
            W1.T.reshape(8, 128, 32, 128).transpose(2, 1, 0, 3).reshape(32, 128, D)
        ).astype(BF),
        "W2Tb": np.ascontiguousarray(W2.T).astype(BF),
        "b1r": np.ascontiguousarray(b1.reshape(32, 128).T),
        "b2": b2.reshape(1, D),
        "g1": g1.reshape(1, D),
        "be1": be1.reshape(1, D),
        "g2": g2.reshape(1, D),
        "be2": be2.reshape(1, D),
    }

    in_maps = []
    for c in range(NCORES):
        b = c // 2
        half = c % 2
        rows = slice(half * NT, (half + 1) * NT)
        # token permutation: own query half first (key/V order is arbitrary
        # as long as srcT and kaug_x agree)
        idx = np.r_[half * NT : (half + 1) * NT, (1 - half) * NT : (2 - half) * NT]
        x = coords[b, :, 0].astype(np.float64)
        y = coords[b, :, 1].astype(np.float64)
        s = (x + y).astype(np.float32)
        thr = np.arange(1, GRID, dtype=np.float64)
        cx = (x[None, :] >= thr[:, None]).astype(np.float32)
        cy = (y[None, :] >= thr[:, None]).astype(np.float32)
        kaug = np.concatenate(
            [s.reshape(1, N), np.zeros((1, N), np.float32), cx, cy], axis=0
        )[:, idx].astype(BF)
        qaug = np.empty((H, 64, NT), np.float32)
        qaug[:, 0, :] = 1.0
        qaug[:, 1, :] = 0.0
        qaug[:, 2:33, :] = -2.0 * cx[None, :, rows]
        qaug[:, 33:64, :] = -2.0 * cy[None, :, rows]
        srcTb = src[b].T
        m = dict(shared)
        m.update(
            {
                "srcT": np.ascontiguousarray(srcTb[:, idx]).astype(BF),
                "src_rows": np.ascontiguousarray(src[b, rows, :]),
                "kaug_x": kaug,
                "qaug_x": qaug.astype(BF),
            }
        )
        in_maps.append(m)
    return in_maps


_NCS = {}
LAST_RUN_S = None


def get_nc(trivial_affine=True):
    if trivial_affine not in _NCS:
        _NCS[trivial_affine] = build_nc(trivial_affine)
    return _NCS[trivial_affine]


def _affine_trivial(inputs):
    return (
        np.all(np.asarray(inputs["g1"]) == 1.0)
        and np.all(np.asarray(inputs["g2"]) == 1.0)
        and not np.any(np.asarray(inputs["be1"]))
        and not np.any(np.asarray(inputs["be2"]))
        and not np.any(np.asarray(inputs["b2"]))
    )


def kernel(**inputs):
    global LAST_RUN_S
    from concourse.bass_utils import run_bass_kernel_spmd

    nc = get_nc(bool(_affine_trivial(inputs)))
    in_maps = host_prep(inputs)
    t0 = time.monotonic()
    res = run_bass_kernel_spmd(nc, in_maps, list(range(NCORES)))
    LAST_RUN_S = time.monotonic() - t0
    full = np.empty((B, N, D), np.float32)
    for c in range(NCORES):
        b = c // 2
        half = c % 2
        full[b, half * NT : (half + 1) * NT, :] = np.asarray(
            res.results[c]["out"]
        ).astype(np.float32)
    return full
